# revision 16
# baseline (speedup 1.0000x reference)
"""Multi-head self-attention Trainium2 kernel (Bass/Tile), v4.

Problem: x:(8,256,32,32), 8 heads, head_dim=32, N=H*W=1024.
Sharding: data-parallel over batch B=8 -> one batch element per NeuronCore.

Per-core math (b fixed, X = x[b] as (C=256, N=1024)):
  q = Wq@X + bq ; k = Wk@X + bk ; v = Wv@X + bv      (per-pixel linear)
  S[n,m] = sum_d q[d,n]k[d,m] / sqrt(32)  (per head)
  P = softmax_m(S) ; O[d,n] = sum_m P[n,m] v[d,m] ; out = Wo@O + bo + X

Bias algebra (exact, folded on host):
  - bk contributes q^T bk, constant along the softmax axis -> drops.
  - bq contributes (bq^T k_raw)[m]: folded as an extra row of the K-hat
    projection (row u_h = Wk_h^T bq_h / sqrt32), matched by a ones-row in
    Q-hat -> scores leave the PE fully biased+scaled.
  - bv contributes bv -> folded into residual via xpb = x[b] + (Wo@bv + bo).
  - 1/sqrt(32) folded into Wq-hat and u rows.

v4 schedule (HW findings: exp on ACT is a ~69us floor; the PE drops to the
mid p-state after ANY idle, doubling matmul time; row-disjoint paired score
matmuls overlap on the PE; fp8 DoubleRow gives no real HW speedup):
  - everything bf16 into the PE; exp ACT ops [128,1024] write bf16 E.
  - flat software pipeline over the 64 (head-pair, m-chunk, n-half) steps:
    per step emit scores(s) [2 overlapped matmuls], exp(s), AV(s-2)
    [2 matmuls, heads column-packed into one PSUM bank]. Projections,
    V-proj, and the normalize/output-projection of finished halves are
    sprinkled in as PE filler so the PE never idles (keeps max p-state)
    and the ACT exp stream never starves.
  - denominators: ones-column in V-hat -> AV emits them for free; per-half
    reciprocal_approx_fast at partition 0 (the custom DVE op is broken at
    nonzero base partitions on HW); recip broadcast via a tiny matmul.
  - tail is jn-split so the last AV / drain / normalize / out-proj / DMA
    chains of the two n-halves overlap.
"""

import math

import numpy as np
import ml_dtypes

import concourse.bass as bass
import concourse.mybir as mybir
import concourse.tile as tile
from concourse import bacc
from concourse.bass_utils import run_bass_kernel_spmd

F32 = mybir.dt.float32
BF16 = mybir.dt.bfloat16
EXP = mybir.ActivationFunctionType.Exp

NH = 8          # heads
HD = 32         # head dim
C = 256         # channels
N = 1024        # H*W
NCORES = 8

BF16NP = np.dtype(ml_dtypes.bfloat16)

DEBUG_DUMPS = False

_NC = None          # cached compiled Bass module
LAST_RESULTS = None  # BassKernelResults of most recent run (for test.py)


def _emit(tc, io):
    nc = tc.nc
    import contextlib

    ctx = contextlib.ExitStack()
    with ctx:
        pers = ctx.enter_context(tc.tile_pool(name="pers", bufs=1))
        etp = ctx.enter_context(tc.tile_pool(name="etp", bufs=4))
        psp = ctx.enter_context(tc.tile_pool(name="psp", bufs=2, space="PSUM"))

        def ptile(name, shape, dtype=F32):
            return pers.tile(shape, dtype, tag=name, name=name)

        # warm the ACT exp table immediately (table PSEUDO_LOAD ~1.3us would
        # otherwise serialize with the first real exp)
        warm = ptile("warm", [1, 8])
        nc.gpsimd.memset(warm[:], 0.0)
        nc.scalar.activation(warm[:], warm[:], EXP)
        wmv = ptile("wmv", [1, 256], BF16)   # moving operand for p-state warmup
        nc.gpsimd.memset(wmv[:], 0.0)

        # ---------------- load inputs ----------------
        X = [ptile(f"X{i}", [128, N], BF16) for i in range(2)]
        XPB = [ptile(f"XPB{i}", [128, N]) for i in range(2)]
        WQT = [ptile(f"WQT{i}", [128, 512], BF16) for i in range(2)]
        WKT = [ptile(f"WKT{i}", [128, 512], BF16) for i in range(2)]
        WVT = [ptile(f"WVT{i}", [128, C], BF16) for i in range(2)]
        WOT = [ptile(f"WOT{i}", [128, C], BF16) for i in range(2)]
        OH = ptile("OH", [4, C], BF16)
        # spread issue across DGE queues: the sync queue alone costs ~565ns
        # per dma_start, serializing the critical X/W loads
        for i in range(2):
            sl = slice(i * 128, (i + 1) * 128)
            nc.sync.dma_start(X[i][:], io["xb"][sl, :])
            nc.sync.dma_start(WQT[i][:], io["wqt"][sl, :])
            nc.scalar.dma_start(WKT[i][:], io["wkt"][sl, :])
            nc.gpsimd.dma_start(WVT[i][:], io["wvt"][sl, :])
            nc.gpsimd.dma_start(WOT[i][:], io["wot"][sl, :])
            nc.gpsimd.dma_start(XPB[i][:], io["xpb"][sl, :])
        nc.gpsimd.dma_start(OH[:], io["oh"][:, :])
        # warm the PE p-state while the DMAs are in flight: back-to-back
        # dummy matmuls (serialized by rewriting one PSUM region) ramp the
        # clock so the first real projections run at full speed
        wps = psp.tile([1, 256], F32, tag="big", bufs=3, name="wps")
        for _ in range(14):
            nc.tensor.matmul(wps[:], warm[:, 0:1].bitcast(BF16)[:, 0:1], wmv[:],
                             start=True, stop=True)

        # ---------------- persistent tiles ----------------
        Qh = [ptile(f"Qh{t}", [128, N], BF16) for t in range(4)]
        Kh = [ptile(f"Kh{t}", [128, N], BF16) for t in range(4)]
        VH = [ptile(f"VH{mc}", [128, NH * 33], BF16) for mc in range(8)]
        O1u = [ptile(f"O1u{t}", [128, N]) for t in range(2)]
        O1 = [ptile(f"O1{t}", [128, N], BF16) for t in range(2)]
        # per-half denominator tiles at base partition 0 (HW quirk: the
        # custom reciprocal_approx_fast op needs base partition 0)
        ESUM = [ptile(f"ESUM{t}", [4, N]) for t in range(2)]
        RECIP = [ptile(f"RECIP{t}", [4, N]) for t in range(2)]
        RECIPB = [ptile(f"RECIPB{t}", [4, N], BF16) for t in range(2)]
        OUTF = [ptile(f"OUTF{t}", [128, N]) for t in range(2)]
        for mc in range(8):
            vh3 = VH[mc].rearrange("p (h c) -> p h c", c=33)
            nc.gpsimd.memset(vh3[:, :, 32:33], 1.0)

        # ---------------- emission helpers ----------------
        def qk_proj(t):
            for dst, w in ((Qh, WQT), (Kh, WKT)):
                pp = psp.tile([128, N], F32, tag="big", bufs=3, name=f"pp_{t}")
                for jn in range(2):
                    for kc in range(2):
                        nc.tensor.matmul(
                            pp[:, jn * 512 : (jn + 1) * 512],
                            w[kc][:, t * 128 : (t + 1) * 128],
                            X[kc][:, jn * 512 : (jn + 1) * 512],
                            start=(kc == 0),
                            stop=(kc == 1),
                        )
                nc.vector.tensor_copy(dst[t][:], pp[:])
            nc.gpsimd.memset(Qh[t][32:33, :], 1.0)
            nc.gpsimd.memset(Qh[t][96:97, :], 1.0)

        def v_proj(mc):
            pv = psp.tile([128, C], F32, tag="big", bufs=3, name=f"pv_{mc}")
            for kc in range(2):
                nc.tensor.matmul(
                    pv[:],
                    X[kc][:, mc * 128 : (mc + 1) * 128],
                    WVT[kc][:],
                    start=(kc == 0),
                    stop=(kc == 1),
                )
            vh3 = VH[mc].rearrange("p (h c) -> p h c", c=33)
            nc.vector.tensor_copy(
                vh3[:, :, 0:32], pv.rearrange("p (h d) -> p h d", d=32)
            )

        psO = [None, None]  # current accumulators, per jn

        def scores(p, mc, jn):
            ps = psp.tile([128, N], F32, tag="big", bufs=3, name=f"ps_{p}_{mc}_{jn}")
            for hh in range(2):  # array rows 0-32 / 64-96 run concurrently
                base = 64 * hh
                nc.tensor.matmul(
                    ps[:, hh * 512 : (hh + 1) * 512],
                    Kh[p][base : base + 33, mc * 128 : (mc + 1) * 128],
                    Qh[p][base : base + 33, jn * 512 : (jn + 1) * 512],
                    start=True,
                    stop=True,
                )
            et = etp.tile([128, N], BF16, tag="et", name=f"et_{p}_{mc}_{jn}")
            nc.scalar.activation(et[:], ps[:], EXP)
            return et

        def av(p, mc, jn, et):
            if psO[jn] is None:
                psO[jn] = psp.tile(
                    [97, 512], F32, tag="psO", bufs=2, name=f"psO_{p}_{jn}"
                )
            for hh in range(2):
                h = 2 * p + hh
                nc.tensor.matmul(
                    psO[jn][64 * hh : 64 * hh + 33, :],
                    VH[mc][:, 33 * h : 33 * h + 33],
                    et[:, hh * 512 : (hh + 1) * 512],
                    start=(mc == 0),
                    stop=(mc == 7),
                    tile_position=(0, 64 * hh),
                    skip_group_check=True,
                )

        def drain(p, jn, tail=False):
            js = slice(jn * 512, (jn + 1) * 512)
            ost = etp.tile([97, 512], F32, tag="ost", bufs=4, name=f"ost_{p}_{jn}")
            if tail:
                # denominators first so the recip chain starts ASAP, and
                # spread the DMAs over queues (sync issue is ~565ns each)
                nc.vector.tensor_copy(ost[32:33, :], psO[jn][32:33, :])
                nc.vector.tensor_copy(ost[96:97, :], psO[jn][96:97, :])
                for hh in range(2):
                    h = 2 * p + hh
                    nc.sync.dma_start(
                        ESUM[h // 4][h % 4 : h % 4 + 1, js],
                        ost[64 * hh + 32 : 64 * hh + 33, :],
                    )
                nc.vector.tensor_copy(ost[0:32, :], psO[jn][0:32, :])
                nc.vector.tensor_copy(ost[64:96, :], psO[jn][64:96, :])
                for hh in range(2):
                    h = 2 * p + hh
                    t, r = h // 4, 32 * (h % 4)
                    q = nc.scalar if hh == 0 else nc.gpsimd
                    q.dma_start(
                        O1u[t][r : r + 32, js], ost[64 * hh : 64 * hh + 32, :]
                    )
            else:
                nc.vector.tensor_copy(ost[0:33, :], psO[jn][0:33, :])
                nc.vector.tensor_copy(ost[64:97, :], psO[jn][64:97, :])
                for hh in range(2):
                    h = 2 * p + hh
                    t, r = h // 4, 32 * (h % 4)
                    nc.sync.dma_start(
                        O1u[t][r : r + 32, js], ost[64 * hh : 64 * hh + 32, :]
                    )
                    nc.sync.dma_start(
                        ESUM[t][h % 4 : h % 4 + 1, js],
                        ost[64 * hh + 32 : 64 * hh + 33, :],
                    )
            psO[jn] = None

        def recip_half(t, jn):
            js = slice(jn * 512, (jn + 1) * 512)
            with nc.allow_low_precision("approx recip of O(100) softmax sums"):
                nc.vector.reciprocal_approx_fast(RECIP[t][:, js], ESUM[t][:, js])
            nc.vector.tensor_copy(RECIPB[t][:, js], RECIP[t][:, js])

        def norm_half(t, jn):
            js = slice(jn * 512, (jn + 1) * 512)
            pr = psp.tile([128, 512], F32, tag="big", bufs=3, name=f"pr_{t}_{jn}")
            nc.tensor.matmul(
                pr[:],
                OH[0:4, t * 128 : (t + 1) * 128],
                RECIPB[t][0:4, js],
                start=True,
                stop=True,
            )
            nc.vector.tensor_mul(O1[t][:, js], O1u[t][:, js], pr[:])

        def oproj(t, mo, jn):
            js = slice(jn * 512, (jn + 1) * 512)
            po = psp.tile([128, 512], F32, tag="big", bufs=3, name=f"po_{t}_{mo}_{jn}")
            nc.tensor.matmul(
                po[:],
                WOT[t][:, mo * 128 : (mo + 1) * 128],
                O1[t][:, js],
                start=True,
                stop=True,
            )
            if t == 0:
                nc.vector.tensor_add(OUTF[mo][:, js], po[:], XPB[mo][:, js])
            else:
                nc.vector.tensor_add(OUTF[mo][:, js], po[:], OUTF[mo][:, js])

        # ---------------- software-pipelined main loop ----------------
        # filler units keep the PE from idling (p-state); qk2/qk3 are
        # reserved for the head-pair boundaries where the AV lag collapses.
        filler = {
            0: lambda: v_proj(2), 2: lambda: v_proj(3), 4: lambda: v_proj(4),
            6: lambda: v_proj(5), 8: lambda: v_proj(6), 10: lambda: v_proj(7),
            12: lambda: qk_proj(1), 15: lambda: qk_proj(2), 31: lambda: qk_proj(3),
        }

        qk_proj(0)
        v_proj(0)
        v_proj(1)

        steps = [(p, mc, jn) for p in range(4) for mc in range(8) for jn in range(2)]
        pend = []  # (p, mc, jn, et) AV work, emitted with lag 2
        for s, (p, mc, jn) in enumerate(steps):
            et = scores(p, mc, jn)
            pend.append((p, mc, jn, et))
            if mc == 0 and jn == 0 and p > 0:
                # head-pair boundary: the new pair's scores were emitted
                # FIRST so the exp stream never waits on the lag collapse
                while len(pend) > 1:
                    av(*pend.pop(0))
                drain(p - 1, 0)
                drain(p - 1, 1)
            elif len(pend) > 2:
                av(*pend.pop(0))
            if s in filler:
                filler.pop(s)()
            # normalize + output-projection of heads 0-3 mid-stream (their
            # drains complete at the p=2 boundary, s=32); spread out so the
            # DVE prerequisites are always long done
            if s == 33:
                recip_half(0, 0)
                recip_half(0, 1)
            elif s == 36:
                norm_half(0, 0)
            elif s == 38:
                norm_half(0, 1)
            elif s == 41:
                oproj(0, 0, 0)
            elif s == 43:
                oproj(0, 0, 1)
            elif s == 45:
                oproj(0, 1, 0)
            elif s == 47:
                oproj(0, 1, 1)

        # ---------------- tail (jn-split, staggered) ----------------
        while len(pend) > 1:
            av(*pend.pop(0))
        drain(3, 0, tail=True)
        recip_half(1, 0)
        av(*pend.pop(0))
        drain(3, 1, tail=True)
        norm_half(1, 0)
        recip_half(1, 1)
        oproj(1, 0, 0)
        norm_half(1, 1)
        nc.sync.dma_start(io["out"][0:128, 0:512], OUTF[0][:, 0:512])
        oproj(1, 1, 0)
        nc.scalar.dma_start(io["out"][128:256, 0:512], OUTF[1][:, 0:512])
        oproj(1, 0, 1)
        nc.scalar.dma_start(io["out"][0:128, 512:1024], OUTF[0][:, 512:1024])
        oproj(1, 1, 1)
        nc.gpsimd.dma_start(io["out"][128:256, 512:1024], OUTF[1][:, 512:1024])

        if DEBUG_DUMPS:
            for nm, t in [
                ("dQh0", Qh[0]), ("dKh0", Kh[0]),
                ("dO1u0", O1u[0]), ("dO1u1", O1u[1]),
                ("dO10", O1[0]), ("dOUTF0", OUTF[0]),
            ]:
                nc.sync.dma_start(io[nm][:, :], t[:])
            for t2 in range(2):
                nc.sync.dma_start(io["dESUM"][4 * t2 : 4 * t2 + 4, :], ESUM[t2][:, :])
                nc.sync.dma_start(io["dRECIP"][4 * t2 : 4 * t2 + 4, :], RECIP[t2][:, :])


def build_nc():
    nc = bacc.Bacc("TRN2", target_bir_lowering=False, debug=False)
    io = {}
    for name, shape, dt_ in [
        ("xb", (C, N), BF16),
        ("xpb", (C, N), F32),
        ("wqt", (C, 512), BF16),
        ("wkt", (C, 512), BF16),
        ("wvt", (C, C), BF16),
        ("wot", (C, C), BF16),
        ("oh", (4, C), BF16),
    ]:
        io[name] = nc.dram_tensor(name, shape, dt_, kind="ExternalInput").ap()
    io["out"] = nc.dram_tensor("out", (C, N), F32, kind="ExternalOutput").ap()
    if DEBUG_DUMPS:
        for nm, shape, dt_ in [
            ("dQh0", (128, N), BF16), ("dKh0", (128, N), BF16),
            ("dESUM", (8, N), F32),
            ("dO1u0", (128, N), F32), ("dO1u1", (128, N), F32),
            ("dRECIP", (8, N), F32), ("dO10", (128, N), BF16),
            ("dOUTF0", (128, N), F32),
        ]:
            io[nm] = nc.dram_tensor(nm, shape, dt_, kind="ExternalOutput").ap()
    with tile.TileContext(nc) as tc:
        _emit(tc, io)
    nc.finalize()  # Bacc passes: wait-splitting (1-wait limit), reg alloc
    return nc


def host_prep(x, Wq, bq, Wk, bk, Wv, bv, Wo, bo):
    """Build per-core input maps (numpy only)."""
    x = np.ascontiguousarray(np.asarray(x, np.float32))
    Wq, bq = np.asarray(Wq, np.float32), np.asarray(bq, np.float32)
    Wk = np.asarray(Wk, np.float32)
    Wv, bv = np.asarray(Wv, np.float32), np.asarray(bv, np.float32)
    Wo, bo = np.asarray(Wo, np.float32), np.asarray(bo, np.float32)
    s = 1.0 / math.sqrt(HD)

    wqt = np.zeros((C, 512), np.float32)
    wkt = np.zeros((C, 512), np.float32)
    for h in range(NH):
        hs = slice(HD * h, HD * (h + 1))
        wqt[:, 64 * h : 64 * h + 32] = Wq[hs, :].T * s
        wkt[:, 64 * h : 64 * h + 32] = Wk[hs, :].T
        wkt[:, 64 * h + 32] = (Wk[hs, :].T @ bq[hs]) * s
    wvt = np.ascontiguousarray(Wv.T)
    wot = np.ascontiguousarray(Wo.T)
    bo2 = Wo @ bv + bo
    # oh[j//32, 128t + j] = 1: broadcasts RECIP row (head index within the
    # half) onto that head's 32 output partitions; same pattern per half.
    oh = np.zeros((4, C), np.float32)
    for t in range(2):
        for j in range(128):
            oh[j // 32, t * 128 + j] = 1.0

    wqt = wqt.astype(BF16NP)
    wkt = wkt.astype(BF16NP)
    wvt = wvt.astype(BF16NP)
    wot = wot.astype(BF16NP)

    B = x.shape[0]
    in_maps = []
    for b in range(B):
        xb = np.ascontiguousarray(x[b].reshape(C, N))
        in_maps.append(
            {
                "xb": xb.astype(BF16NP),
                "xpb": np.ascontiguousarray(xb + bo2[:, None]),
                "wqt": wqt,
                "wkt": wkt,
                "wvt": wvt,
                "wot": wot,
                "oh": oh.astype(BF16NP),
            }
        )
    return in_maps


def kernel(x, Wq, bq, Wk, bk, Wv, bv, Wo, bo):
    global _NC, LAST_RESULTS
    if _NC is None:
        _NC = build_nc()
    in_maps = host_prep(x, Wq, bq, Wk, bk, Wv, bv, Wo, bo)
    res = run_bass_kernel_spmd(_NC, in_maps, core_ids=list(range(NCORES)))
    LAST_RESULTS = res
    out = np.stack([r["out"] for r in res.results], axis=0)
    return out.reshape(NCORES, C, 32, 32).astype(np.float32)


if __name__ == "__main__":
    # smoke: random inputs through the kernel
    rng = np.random.default_rng(0)
    ins = {
        "x": rng.standard_normal((8, C, 32, 32), dtype=np.float32),
        "Wq": rng.standard_normal((C, C), dtype=np.float32) / 16,
        "bq": rng.standard_normal(C).astype(np.float32) * 0.01,
        "Wk": rng.standard_normal((C, C), dtype=np.float32) / 16,
        "bk": rng.standard_normal(C).astype(np.float32) * 0.01,
        "Wv": rng.standard_normal((C, C), dtype=np.float32) / 16,
        "bv": rng.standard_normal(C).astype(np.float32) * 0.01,
        "Wo": rng.standard_normal((C, C), dtype=np.float32) / 16,
        "bo": rng.standard_normal(C).astype(np.float32) * 0.01,
    }
    out = kernel(**ins)
    print("out", out.shape, out.dtype, float(np.abs(out).mean()))


# revision 17
# speedup vs baseline: 1.0417x; 1.0417x over previous
"""Multi-head self-attention Trainium2 kernel (Bass/Tile), v4.

Problem: x:(8,256,32,32), 8 heads, head_dim=32, N=H*W=1024.
Sharding: data-parallel over batch B=8 -> one batch element per NeuronCore.

Per-core math (b fixed, X = x[b] as (C=256, N=1024)):
  q = Wq@X + bq ; k = Wk@X + bk ; v = Wv@X + bv      (per-pixel linear)
  S[n,m] = sum_d q[d,n]k[d,m] / sqrt(32)  (per head)
  P = softmax_m(S) ; O[d,n] = sum_m P[n,m] v[d,m] ; out = Wo@O + bo + X

Bias algebra (exact, folded on host):
  - bk contributes q^T bk, constant along the softmax axis -> drops.
  - bq contributes (bq^T k_raw)[m]: folded as an extra row of the K-hat
    projection (row u_h = Wk_h^T bq_h / sqrt32), matched by a ones-row in
    Q-hat -> scores leave the PE fully biased+scaled.
  - bv contributes bv -> folded into residual via xpb = x[b] + (Wo@bv + bo).
  - 1/sqrt(32) folded into Wq-hat and u rows.

v4 schedule (HW findings: exp on ACT is a ~69us floor; the PE drops to the
mid p-state after ANY idle, doubling matmul time; row-disjoint paired score
matmuls overlap on the PE; fp8 DoubleRow gives no real HW speedup):
  - everything bf16 into the PE; exp ACT ops [128,1024] write bf16 E.
  - flat software pipeline over the 64 (head-pair, m-chunk, n-half) steps:
    per step emit scores(s) [2 overlapped matmuls], exp(s), AV(s-2)
    [2 matmuls, heads column-packed into one PSUM bank]. Projections,
    V-proj, and the normalize/output-projection of finished halves are
    sprinkled in as PE filler so the PE never idles (keeps max p-state)
    and the ACT exp stream never starves.
  - denominators: ones-column in V-hat -> AV emits them for free; per-half
    reciprocal_approx_fast at partition 0 (the custom DVE op is broken at
    nonzero base partitions on HW); recip broadcast via a tiny matmul.
  - tail is jn-split so the last AV / drain / normalize / out-proj / DMA
    chains of the two n-halves overlap.
"""

import math

import numpy as np
import ml_dtypes

import concourse.bass as bass
import concourse.mybir as mybir
import concourse.tile as tile
from concourse import bacc
from concourse.bass_utils import run_bass_kernel_spmd

F32 = mybir.dt.float32
BF16 = mybir.dt.bfloat16
EXP = mybir.ActivationFunctionType.Exp

NH = 8          # heads
HD = 32         # head dim
C = 256         # channels
N = 1024        # H*W
NCORES = 8

BF16NP = np.dtype(ml_dtypes.bfloat16)

DEBUG_DUMPS = False

_NC = None          # cached compiled Bass module
LAST_RESULTS = None  # BassKernelResults of most recent run (for test.py)


def _emit(tc, io):
    nc = tc.nc
    import contextlib

    ctx = contextlib.ExitStack()
    with ctx:
        pers = ctx.enter_context(tc.tile_pool(name="pers", bufs=1))
        etp = ctx.enter_context(tc.tile_pool(name="etp", bufs=4))
        psp = ctx.enter_context(tc.tile_pool(name="psp", bufs=2, space="PSUM"))

        def ptile(name, shape, dtype=F32):
            return pers.tile(shape, dtype, tag=name, name=name)

        # warm the ACT exp table immediately (table PSEUDO_LOAD ~1.3us would
        # otherwise serialize with the first real exp)
        warm = ptile("warm", [1, 8])
        nc.gpsimd.memset(warm[:], 0.0)
        nc.scalar.activation(warm[:], warm[:], EXP)
        wmv = ptile("wmv", [1, 256], BF16)   # moving operand for p-state warmup
        nc.gpsimd.memset(wmv[:], 0.0)

        # ---------------- load inputs ----------------
        X = [ptile(f"X{i}", [128, N], BF16) for i in range(2)]
        XPB = [ptile(f"XPB{i}", [128, N]) for i in range(2)]
        WQT = [ptile(f"WQT{i}", [128, 512], BF16) for i in range(2)]
        WKT = [ptile(f"WKT{i}", [128, 512], BF16) for i in range(2)]
        WVT = [ptile(f"WVT{i}", [128, C], BF16) for i in range(2)]
        WOT = [ptile(f"WOT{i}", [128, C], BF16) for i in range(2)]
        OH = ptile("OH", [4, C], BF16)
        # spread issue across DGE queues (sync costs ~565ns per dma_start);
        # X tiles first -- the first projections need both
        for i in range(2):
            nc.sync.dma_start(X[i][:], io["xb"][i * 128 : (i + 1) * 128, :])
        for i in range(2):
            sl = slice(i * 128, (i + 1) * 128)
            nc.scalar.dma_start(WQT[i][:], io["wqt"][sl, :])
            nc.scalar.dma_start(WKT[i][:], io["wkt"][sl, :])
            nc.gpsimd.dma_start(WVT[i][:], io["wvt"][sl, :])
            nc.gpsimd.dma_start(WOT[i][:], io["wot"][sl, :])
            nc.sync.dma_start(XPB[i][:], io["xpb"][sl, :])
        nc.gpsimd.dma_start(OH[:], io["oh"][:, :])
        # warm the PE p-state while the DMAs are in flight
        wps = psp.tile([1, 256], F32, tag="big", bufs=3, name="wps")
        for _ in range(6):
            nc.tensor.matmul(wps[:], warm[:, 0:1].bitcast(BF16)[:, 0:1], wmv[:],
                             start=True, stop=True)

        # ---------------- persistent tiles ----------------
        Qh = [ptile(f"Qh{t}", [128, N], BF16) for t in range(4)]
        Kh = [ptile(f"Kh{t}", [128, N], BF16) for t in range(4)]
        VH = [ptile(f"VH{mc}", [128, NH * 33], BF16) for mc in range(8)]
        O1u = [ptile(f"O1u{t}", [128, N]) for t in range(2)]
        O1 = [ptile(f"O1{t}", [128, N], BF16) for t in range(2)]
        # per-half denominator tiles at base partition 0 (HW quirk: the
        # custom reciprocal_approx_fast op needs base partition 0)
        ESUM = [ptile(f"ESUM{t}", [4, N]) for t in range(2)]
        RECIP = [ptile(f"RECIP{t}", [4, N]) for t in range(2)]
        RECIPB = [ptile(f"RECIPB{t}", [4, N], BF16) for t in range(2)]
        OUTF = [ptile(f"OUTF{t}", [128, N]) for t in range(2)]
        for mc in range(8):
            vh3 = VH[mc].rearrange("p (h c) -> p h c", c=33)
            nc.gpsimd.memset(vh3[:, :, 32:33], 1.0)

        # ---------------- emission helpers ----------------
        def proj_mm(t, w, name):
            pp = psp.tile([128, N], F32, tag="big", bufs=3, name=name)
            for jn in range(2):
                for kc in range(2):
                    nc.tensor.matmul(
                        pp[:, jn * 512 : (jn + 1) * 512],
                        w[kc][:, t * 128 : (t + 1) * 128],
                        X[kc][:, jn * 512 : (jn + 1) * 512],
                        start=(kc == 0),
                        stop=(kc == 1),
                    )
            return pp

        def q_unit(t):
            pp = proj_mm(t, WQT, f"ppq_{t}")
            nc.vector.tensor_copy(Qh[t][:], pp[:])
            nc.gpsimd.memset(Qh[t][32:33, :], 1.0)
            nc.gpsimd.memset(Qh[t][96:97, :], 1.0)

        def k_unit(t):
            pp = proj_mm(t, WKT, f"ppk_{t}")
            nc.vector.tensor_copy(Kh[t][:], pp[:])

        def qk0_fast():
            # first tile: cast/ones in 512-col halves so the first scores
            # wait only on the jn0 half
            ppq = proj_mm(0, WQT, "ppq_0")
            ppk = proj_mm(0, WKT, "ppk_0")
            for half in range(2):
                hs = slice(512 * half, 512 * half + 512)
                nc.vector.tensor_copy(Kh[0][:, hs], ppk[:, hs])
                nc.vector.tensor_copy(Qh[0][:, hs], ppq[:, hs])
                nc.gpsimd.memset(Qh[0][32:33, hs], 1.0)
                nc.gpsimd.memset(Qh[0][96:97, hs], 1.0)

        def v_proj(mc):
            pv = psp.tile([128, C], F32, tag="big", bufs=3, name=f"pv_{mc}")
            for kc in range(2):
                nc.tensor.matmul(
                    pv[:],
                    X[kc][:, mc * 128 : (mc + 1) * 128],
                    WVT[kc][:],
                    start=(kc == 0),
                    stop=(kc == 1),
                )
            vh3 = VH[mc].rearrange("p (h c) -> p h c", c=33)
            nc.vector.tensor_copy(
                vh3[:, :, 0:32], pv.rearrange("p (h d) -> p h d", d=32)
            )

        psO = [None, None]  # current accumulators, per jn

        def scores(p, mc, jn):
            ps = psp.tile([128, N], F32, tag="big", bufs=3, name=f"ps_{p}_{mc}_{jn}")
            for hh in range(2):  # array rows 0-32 / 64-96 run concurrently
                base = 64 * hh
                nc.tensor.matmul(
                    ps[:, hh * 512 : (hh + 1) * 512],
                    Kh[p][base : base + 33, mc * 128 : (mc + 1) * 128],
                    Qh[p][base : base + 33, jn * 512 : (jn + 1) * 512],
                    start=True,
                    stop=True,
                )
            et = etp.tile([128, N], BF16, tag="et", name=f"et_{p}_{mc}_{jn}")
            nc.scalar.activation(et[:], ps[:], EXP)
            return et

        def av(p, mc, jn, et):
            if psO[jn] is None:
                psO[jn] = psp.tile(
                    [97, 512], F32, tag="psO", bufs=2, name=f"psO_{p}_{jn}"
                )
            for hh in range(2):
                h = 2 * p + hh
                nc.tensor.matmul(
                    psO[jn][64 * hh : 64 * hh + 33, :],
                    VH[mc][:, 33 * h : 33 * h + 33],
                    et[:, hh * 512 : (hh + 1) * 512],
                    start=(mc == 0),
                    stop=(mc == 7),
                    tile_position=(0, 64 * hh),
                    skip_group_check=True,
                )

        def drain(p, jn):
            js = slice(jn * 512, (jn + 1) * 512)
            ost = etp.tile([97, 512], F32, tag="ost", bufs=4, name=f"ost_{p}_{jn}")
            nc.vector.tensor_copy(ost[0:33, :], psO[jn][0:33, :])
            nc.vector.tensor_copy(ost[64:97, :], psO[jn][64:97, :])
            for hh in range(2):
                h = 2 * p + hh
                t, r = h // 4, 32 * (h % 4)
                nc.sync.dma_start(
                    O1u[t][r : r + 32, js], ost[64 * hh : 64 * hh + 32, :]
                )
                nc.sync.dma_start(
                    ESUM[t][h % 4 : h % 4 + 1, js],
                    ost[64 * hh + 32 : 64 * hh + 33, :],
                )
            psO[jn] = None

        def recip_half(t, jn):
            js = slice(jn * 512, (jn + 1) * 512)
            with nc.allow_low_precision("approx recip of O(100) softmax sums"):
                nc.vector.reciprocal_approx_fast(RECIP[t][:, js], ESUM[t][:, js])
            nc.vector.tensor_copy(RECIPB[t][:, js], RECIP[t][:, js])

        def norm_half(t, jn):
            js = slice(jn * 512, (jn + 1) * 512)
            pr = psp.tile([128, 512], F32, tag="big", bufs=3, name=f"pr_{t}_{jn}")
            nc.tensor.matmul(
                pr[:],
                OH[0:4, t * 128 : (t + 1) * 128],
                RECIPB[t][0:4, js],
                start=True,
                stop=True,
            )
            nc.vector.tensor_mul(O1[t][:, js], O1u[t][:, js], pr[:])

        def oproj(t, mo, jn):
            js = slice(jn * 512, (jn + 1) * 512)
            po = psp.tile([128, 512], F32, tag="big", bufs=3, name=f"po_{t}_{mo}_{jn}")
            nc.tensor.matmul(
                po[:],
                WOT[t][:, mo * 128 : (mo + 1) * 128],
                O1[t][:, js],
                start=True,
                stop=True,
            )
            if t == 0:
                nc.vector.tensor_add(OUTF[mo][:, js], po[:], XPB[mo][:, js])
            else:
                nc.vector.tensor_add(OUTF[mo][:, js], po[:], OUTF[mo][:, js])

        # ---------------- software-pipelined main loop ----------------
        # fine-grained filler units keep the PE from idling (p-state)
        # without ever monopolizing it long enough to starve the ACT
        filler = {
            0: lambda: v_proj(2), 2: lambda: v_proj(3), 4: lambda: v_proj(4),
            6: lambda: v_proj(5), 8: lambda: v_proj(6), 10: lambda: v_proj(7),
            12: lambda: q_unit(1), 14: lambda: k_unit(1),
            18: lambda: q_unit(2), 20: lambda: k_unit(2),
            24: lambda: q_unit(3), 26: lambda: k_unit(3),
        }

        qk0_fast()
        v_proj(0)
        v_proj(1)

        steps = [(p, mc, jn) for p in range(4) for mc in range(8) for jn in range(2)]
        pend = []  # (p, mc, jn, et) AV work, emitted with lag 2
        for s, (p, mc, jn) in enumerate(steps):
            et = scores(p, mc, jn)
            pend.append((p, mc, jn, et))
            if mc == 0 and jn == 0 and p > 0:
                # head-pair boundary: the new pair's scores were emitted
                # FIRST so the exp stream never waits on the lag collapse
                while len(pend) > 1:
                    av(*pend.pop(0))
                drain(p - 1, 0)
                drain(p - 1, 1)
            elif len(pend) > 2:
                av(*pend.pop(0))
            if s in filler:
                filler.pop(s)()
            # normalize + output-projection of heads 0-3 mid-stream (their
            # drains complete at the p=2 boundary, s=32); spread out so the
            # DVE prerequisites are always long done
            if s == 33:
                recip_half(0, 0)
                recip_half(0, 1)
            elif s == 36:
                norm_half(0, 0)
            elif s == 38:
                norm_half(0, 1)
            elif s == 41:
                oproj(0, 0, 0)
            elif s == 43:
                oproj(0, 0, 1)
            elif s == 45:
                oproj(0, 1, 0)
            elif s == 47:
                oproj(0, 1, 1)

        # ---------------- tail (jn-split, staggered) ----------------
        # heads 6,7: ACT (idle after the last exp) does the jn0 drain copies
        # while the DVE does jn1's; ESUM rows DMA first to start the recip
        # chain; everything jn-split so the two chains overlap.
        av(*pend.pop(0))
        ost0 = etp.tile([97, 512], F32, tag="ost", bufs=4, name="ost_t0")
        nc.scalar.copy(ost0[0:33, :], psO[0][0:33, :])
        nc.scalar.copy(ost0[64:97, :], psO[0][64:97, :])
        psO[0] = None
        nc.sync.dma_start(ESUM[1][2:3, 0:512], ost0[32:33, :])
        nc.sync.dma_start(ESUM[1][3:4, 0:512], ost0[96:97, :])
        nc.sync.dma_start(O1u[1][64:96, 0:512], ost0[0:32, :])
        nc.sync.dma_start(O1u[1][96:128, 0:512], ost0[64:96, :])
        av(*pend.pop(0))
        ost1 = etp.tile([97, 512], F32, tag="ost", bufs=4, name="ost_t1")
        nc.vector.tensor_copy(ost1[0:33, :], psO[1][0:33, :])
        nc.vector.tensor_copy(ost1[64:97, :], psO[1][64:97, :])
        psO[1] = None
        nc.scalar.dma_start(ESUM[1][2:3, 512:1024], ost1[32:33, :])
        nc.scalar.dma_start(ESUM[1][3:4, 512:1024], ost1[96:97, :])
        nc.scalar.dma_start(O1u[1][64:96, 512:1024], ost1[0:32, :])
        nc.scalar.dma_start(O1u[1][96:128, 512:1024], ost1[64:96, :])
        recip_half(1, 0)
        norm_half(1, 0)
        recip_half(1, 1)
        oproj(1, 0, 0)
        norm_half(1, 1)
        nc.sync.dma_start(io["out"][0:128, 0:512], OUTF[0][:, 0:512])
        oproj(1, 1, 0)
        nc.sync.dma_start(io["out"][128:256, 0:512], OUTF[1][:, 0:512])
        oproj(1, 0, 1)
        nc.scalar.dma_start(io["out"][0:128, 512:1024], OUTF[0][:, 512:1024])
        oproj(1, 1, 1)
        nc.scalar.dma_start(io["out"][128:256, 512:1024], OUTF[1][:, 512:1024])

        if DEBUG_DUMPS:
            for nm, t in [
                ("dQh0", Qh[0]), ("dKh0", Kh[0]),
                ("dO1u0", O1u[0]), ("dO1u1", O1u[1]),
                ("dO10", O1[0]), ("dOUTF0", OUTF[0]),
            ]:
                nc.sync.dma_start(io[nm][:, :], t[:])
            for t2 in range(2):
                nc.sync.dma_start(io["dESUM"][4 * t2 : 4 * t2 + 4, :], ESUM[t2][:, :])
                nc.sync.dma_start(io["dRECIP"][4 * t2 : 4 * t2 + 4, :], RECIP[t2][:, :])


def build_nc():
    nc = bacc.Bacc("TRN2", target_bir_lowering=False, debug=False)
    io = {}
    for name, shape, dt_ in [
        ("xb", (C, N), BF16),
        ("xpb", (C, N), F32),
        ("wqt", (C, 512), BF16),
        ("wkt", (C, 512), BF16),
        ("wvt", (C, C), BF16),
        ("wot", (C, C), BF16),
        ("oh", (4, C), BF16),
    ]:
        io[name] = nc.dram_tensor(name, shape, dt_, kind="ExternalInput").ap()
    io["out"] = nc.dram_tensor("out", (C, N), F32, kind="ExternalOutput").ap()
    if DEBUG_DUMPS:
        for nm, shape, dt_ in [
            ("dQh0", (128, N), BF16), ("dKh0", (128, N), BF16),
            ("dESUM", (8, N), F32),
            ("dO1u0", (128, N), F32), ("dO1u1", (128, N), F32),
            ("dRECIP", (8, N), F32), ("dO10", (128, N), BF16),
            ("dOUTF0", (128, N), F32),
        ]:
            io[nm] = nc.dram_tensor(nm, shape, dt_, kind="ExternalOutput").ap()
    with tile.TileContext(nc) as tc:
        _emit(tc, io)
    nc.finalize()  # Bacc passes: wait-splitting (1-wait limit), reg alloc
    return nc


def host_prep(x, Wq, bq, Wk, bk, Wv, bv, Wo, bo):
    """Build per-core input maps (numpy only)."""
    x = np.ascontiguousarray(np.asarray(x, np.float32))
    Wq, bq = np.asarray(Wq, np.float32), np.asarray(bq, np.float32)
    Wk = np.asarray(Wk, np.float32)
    Wv, bv = np.asarray(Wv, np.float32), np.asarray(bv, np.float32)
    Wo, bo = np.asarray(Wo, np.float32), np.asarray(bo, np.float32)
    s = 1.0 / math.sqrt(HD)

    wqt = np.zeros((C, 512), np.float32)
    wkt = np.zeros((C, 512), np.float32)
    for h in range(NH):
        hs = slice(HD * h, HD * (h + 1))
        wqt[:, 64 * h : 64 * h + 32] = Wq[hs, :].T * s
        wkt[:, 64 * h : 64 * h + 32] = Wk[hs, :].T
        wkt[:, 64 * h + 32] = (Wk[hs, :].T @ bq[hs]) * s
    wvt = np.ascontiguousarray(Wv.T)
    wot = np.ascontiguousarray(Wo.T)
    bo2 = Wo @ bv + bo
    # oh[j//32, 128t + j] = 1: broadcasts RECIP row (head index within the
    # half) onto that head's 32 output partitions; same pattern per half.
    oh = np.zeros((4, C), np.float32)
    for t in range(2):
        for j in range(128):
            oh[j // 32, t * 128 + j] = 1.0

    wqt = wqt.astype(BF16NP)
    wkt = wkt.astype(BF16NP)
    wvt = wvt.astype(BF16NP)
    wot = wot.astype(BF16NP)

    B = x.shape[0]
    in_maps = []
    for b in range(B):
        xb = np.ascontiguousarray(x[b].reshape(C, N))
        in_maps.append(
            {
                "xb": xb.astype(BF16NP),
                "xpb": np.ascontiguousarray(xb + bo2[:, None]),
                "wqt": wqt,
                "wkt": wkt,
                "wvt": wvt,
                "wot": wot,
                "oh": oh.astype(BF16NP),
            }
        )
    return in_maps


def kernel(x, Wq, bq, Wk, bk, Wv, bv, Wo, bo):
    global _NC, LAST_RESULTS
    if _NC is None:
        _NC = build_nc()
    in_maps = host_prep(x, Wq, bq, Wk, bk, Wv, bv, Wo, bo)
    res = run_bass_kernel_spmd(_NC, in_maps, core_ids=list(range(NCORES)))
    LAST_RESULTS = res
    out = np.stack([r["out"] for r in res.results], axis=0)
    return out.reshape(NCORES, C, 32, 32).astype(np.float32)


if __name__ == "__main__":
    # smoke: random inputs through the kernel
    rng = np.random.default_rng(0)
    ins = {
        "x": rng.standard_normal((8, C, 32, 32), dtype=np.float32),
        "Wq": rng.standard_normal((C, C), dtype=np.float32) / 16,
        "bq": rng.standard_normal(C).astype(np.float32) * 0.01,
        "Wk": rng.standard_normal((C, C), dtype=np.float32) / 16,
        "bk": rng.standard_normal(C).astype(np.float32) * 0.01,
        "Wv": rng.standard_normal((C, C), dtype=np.float32) / 16,
        "bv": rng.standard_normal(C).astype(np.float32) * 0.01,
        "Wo": rng.standard_normal((C, C), dtype=np.float32) / 16,
        "bo": rng.standard_normal(C).astype(np.float32) * 0.01,
    }
    out = kernel(**ins)
    print("out", out.shape, out.dtype, float(np.abs(out).mean()))
